# revision 10
# baseline (speedup 1.0000x reference)
"""LSTMCell on 8 Trainium2 NeuronCores, data-parallel over the batch.

Full inputs: x/h_t/c_t [65536,128] f32, 8 gate weight matrices [128,128],
4 biases [128]. Returns (h_new, c_new) as [65536,128] f32 each.

Per core (8192 rows): batch tiles of 128 rows, groups of 4 tiles.
  - PE transposes x/h tiles (fp32) into PSUM, DVE copies them to SBUF
    rounding to f32r.
  - Two f32r matmuls per tile accumulate gates [128 batch, 512] into one
    PSUM bank; 4 tiles share a [128,2048] 4-bank "quad" tile.
  - Gate order [i, f, o, g] with W_g,b_g pre-scaled by 2 on host, so ONE
    sigmoid over the whole quad computes i,f,o and s=sigmoid(2g_a);
    tanh(g_a) = 2s-1 is a fused DVE tensor_scalar.
  - c_new = f*c + i*g on DVE; tanh(c_new) per group on ACT; h_new = o*tanh.
"""
import numpy as np
from contextlib import ExitStack

import concourse.bass as bass
import concourse.tile as tile
from concourse import bacc, mybir
from concourse.bass_utils import run_bass_kernel_spmd
from concourse.masks import make_identity

F32 = mybir.dt.float32
F32R = mybir.dt.float32r
AF = mybir.ActivationFunctionType
ALU = mybir.AluOpType

NCORES = 8
BC = 8192            # batch rows per core
GROUP_ROWS = 512     # 4 tiles of 128
NT = 4               # tiles per group
NG = BC // GROUP_ROWS

_CACHE = {}


def _build(has_bias: bool):
    nc = bacc.Bacc("TRN2", target_bir_lowering=False, debug=False)
    x = nc.dram_tensor("x", [BC, 128], F32, kind="ExternalInput").ap()
    h = nc.dram_tensor("h", [BC, 128], F32, kind="ExternalInput").ap()
    c = nc.dram_tensor("c", [BC, 128], F32, kind="ExternalInput").ap()
    wxt = nc.dram_tensor("wxt", [128, 512], F32R, kind="ExternalInput").ap()
    wht = nc.dram_tensor("wht", [128, 512], F32R, kind="ExternalInput").ap()
    if has_bias:
        bias = nc.dram_tensor("bias", [1, 512], F32R, kind="ExternalInput").ap()
    hn = nc.dram_tensor("hn", [BC, 128], F32, kind="ExternalOutput").ap()
    cn = nc.dram_tensor("cn", [BC, 128], F32, kind="ExternalOutput").ap()

    with tile.TileContext(nc) as tc:
        with ExitStack() as ctx:
            const = ctx.enter_context(tc.tile_pool(name="const", bufs=1))
            inp = ctx.enter_context(tc.tile_pool(name="inp", bufs=4))
            xht = ctx.enter_context(tc.tile_pool(name="xht", bufs=8))
            qp = ctx.enter_context(tc.tile_pool(name="qp", bufs=2, space="PSUM"))
            sp = ctx.enter_context(tc.tile_pool(name="sp", bufs=3))
            op = ctx.enter_context(tc.tile_pool(name="op", bufs=4))
            tmp = ctx.enter_context(tc.tile_pool(name="tmp", bufs=6))

            ident = const.tile([128, 128], F32)
            make_identity(nc, ident)
            wx_sb = const.tile([128, 512], F32R)
            nc.sync.dma_start(wx_sb[:], wxt)
            wh_sb = const.tile([128, 512], F32R)
            nc.sync.dma_start(wh_sb[:], wht)
            if has_bias:
                ones = const.tile([1, 128], F32R)
                nc.vector.memset(ones[:], 1.0)
                b_sb = const.tile([1, 512], F32R)
                nc.sync.dma_start(b_sb[:], bias)

            warm = qp.tile([128, 2048], F32, name="warm", tag="quad")
            for _ in range(16):
                nc.tensor.matmul(warm[:, 0:128], ident[:], ident[:],
                                 is_transpose=True, start=True, stop=True)

            for g in range(NG):
                r0 = g * GROUP_ROWS
                xg = inp.tile([128, GROUP_ROWS], F32, name=f"xg{g}", tag="xg")
                hg = inp.tile([128, GROUP_ROWS], F32, name=f"hg{g}", tag="hg")
                cg = inp.tile([128, GROUP_ROWS], F32, name=f"cg{g}", tag="cg")
                for sb_t, dram in ((xg, x), (hg, h), (cg, c)):
                    nc.sync.dma_start(
                        sb_t[:].rearrange("p (t f) -> p t f", t=NT),
                        dram[r0:r0 + GROUP_ROWS, :].rearrange(
                            "(t p) f -> p t f", p=128))

                quad = qp.tile([128, 2048], F32, name=f"quad{g}", tag="quad")
                # pass A: all transposes back-to-back on PE, then ONE wide
                # rounding cast over all 4 banks (strided 3D AP)
                for t in range(NT):
                    col = t * 512
                    fs = t * 128
                    nc.tensor.matmul(quad[:, col:col + 128],
                                     xg[:, fs:fs + 128], ident[:],
                                     is_transpose=True, start=True, stop=False)
                    nc.tensor.matmul(quad[:, col + 128:col + 256],
                                     hg[:, fs:fs + 128], ident[:],
                                     is_transpose=True, start=False, stop=True)
                xh_w = xht.tile([128, 1024], F32R, name=f"xh{g}", tag="xh")
                nc.vector.tensor_copy(
                    xh_w[:].rearrange("p (t x) -> p t x", t=NT),
                    quad[:].rearrange("p (t x) -> p t x", t=NT)[:, :, 0:256])
                # pass B: all gates matmuls = [x h] @ [WxT; WhT] (+ bias)
                for t in range(NT):
                    col = t * 512
                    xh = xh_w[:, t * 256:(t + 1) * 256]
                    first = True
                    if has_bias:
                        nc.tensor.matmul(quad[:, col:col + 512], ones[:],
                                         b_sb[:], start=True, stop=False)
                        first = False
                    nc.tensor.matmul(quad[:, col:col + 512], xh[:, 0:128],
                                     wx_sb[:], start=first, stop=False)
                    nc.tensor.matmul(quad[:, col:col + 512], xh[:, 128:256],
                                     wh_sb[:], start=False, stop=True)

                sig = sp.tile([128, 2048], F32, name=f"sig{g}", tag="sig")
                cn_g = op.tile([128, GROUP_ROWS], F32, name=f"cn{g}", tag="cn")
                tc_g = op.tile([128, GROUP_ROWS], F32, name=f"tc{g}", tag="tcg")
                hn_g = op.tile([128, GROUP_ROWS], F32, name=f"hn{g}", tag="hn")
                # one sigmoid over the whole quad (4 banks), then wide DVE
                # ops via 3D (p, t, x) access patterns over all 4 tiles.
                nc.scalar.activation(sig[:], quad[:], AF.Sigmoid)
                sig3 = sig[:].rearrange("p (t x) -> p t x", t=NT)
                i_ap = sig3[:, :, 0:128]
                f_ap = sig3[:, :, 128:256]
                o_ap = sig3[:, :, 256:384]
                s_ap = sig3[:, :, 384:512]
                c3 = cg[:].rearrange("p (t x) -> p t x", t=NT)
                gt = tmp.tile([128, 512], F32, name=f"gt{g}", tag="gt")
                gt3 = gt[:].rearrange("p (t x) -> p t x", t=NT)
                nc.vector.tensor_scalar(gt3, s_ap, 2.0, 1.0,
                                        ALU.mult, ALU.subtract)
                ig = tmp.tile([128, 512], F32, name=f"ig{g}", tag="ig")
                ig3 = ig[:].rearrange("p (t x) -> p t x", t=NT)
                nc.vector.tensor_mul(ig3, i_ap, gt3)
                fc = tmp.tile([128, 512], F32, name=f"fc{g}", tag="fc")
                fc3 = fc[:].rearrange("p (t x) -> p t x", t=NT)
                nc.vector.tensor_mul(fc3, f_ap, c3)
                nc.vector.tensor_add(cn_g[:], ig[:], fc[:])
                nc.scalar.activation(tc_g[:], cn_g[:], AF.Tanh)
                tc3 = tc_g[:].rearrange("p (t x) -> p t x", t=NT)
                hn3 = hn_g[:].rearrange("p (t x) -> p t x", t=NT)
                # gpsimd is otherwise idle; h_new mul is off the critical path
                nc.gpsimd.tensor_mul(hn3, o_ap, tc3)
                for sb_t, dram in ((hn_g, hn), (cn_g, cn)):
                    nc.sync.dma_start(
                        dram[r0:r0 + GROUP_ROWS, :].rearrange(
                            "(t p) f -> p t f", p=128),
                        sb_t[:].rearrange("p (t f) -> p t f", t=NT))
    nc.compile()
    return nc


def _run(inputs, trace=False, tmpdir=None):
    x = np.ascontiguousarray(inputs["x"], dtype=np.float32)
    h = np.ascontiguousarray(inputs["h_t"], dtype=np.float32)
    c = np.ascontiguousarray(inputs["c_t"], dtype=np.float32)
    # gate order [i, f, o, g]; W_g/b_g scaled by 2 for the tanh-via-sigmoid
    wx = np.concatenate([inputs["W_ii"], inputs["W_if"], inputs["W_io"],
                         2.0 * np.asarray(inputs["W_ig"])], axis=0)
    wh = np.concatenate([inputs["W_hi"], inputs["W_hf"], inputs["W_ho"],
                         2.0 * np.asarray(inputs["W_hg"])], axis=0)
    b = np.concatenate([inputs["b_i"], inputs["b_f"], inputs["b_o"],
                        2.0 * np.asarray(inputs["b_g"])], axis=0)
    wxt = np.ascontiguousarray(wx.T, dtype=np.float32)
    wht = np.ascontiguousarray(wh.T, dtype=np.float32)
    has_bias = bool(np.any(b))

    key = has_bias
    if key not in _CACHE:
        _CACHE[key] = _build(has_bias)
    nc = _CACHE[key]

    in_maps = []
    for i in range(NCORES):
        m = {
            "x": x[i * BC:(i + 1) * BC],
            "h": h[i * BC:(i + 1) * BC],
            "c": c[i * BC:(i + 1) * BC],
            "wxt": wxt,
            "wht": wht,
        }
        if has_bias:
            m["bias"] = b.reshape(1, 512).astype(np.float32)
        in_maps.append(m)

    res = run_bass_kernel_spmd(nc, in_maps, core_ids=list(range(NCORES)),
                               trace=trace, tmpdir=tmpdir)
    h_new = np.concatenate([r["hn"] for r in res.results], axis=0)
    c_new = np.concatenate([r["cn"] for r in res.results], axis=0)
    return h_new, c_new, res


def kernel(**inputs):
    h_new, c_new, _ = _run(inputs, trace=False)
    return h_new, c_new


# revision 13
# speedup vs baseline: 1.0624x; 1.0624x over previous
"""LSTMCell on 8 Trainium2 NeuronCores, data-parallel over the batch.

Full inputs: x/h_t/c_t [65536,128] f32, 8 gate weight matrices [128,128],
4 biases [128]. Returns (h_new, c_new) as [65536,128] f32 each.

Per core (8192 rows): batch tiles of 128 rows, groups of 4 tiles.
  - PE transposes x/h tiles (fp32) into PSUM, DVE copies them to SBUF
    rounding to f32r.
  - Two f32r matmuls per tile accumulate gates [128 batch, 512] into one
    PSUM bank; 4 tiles share a [128,2048] 4-bank "quad" tile.
  - Gate order [i, f, o, g] with W_g,b_g pre-scaled by 2 on host, so ONE
    sigmoid over the whole quad computes i,f,o and s=sigmoid(2g_a);
    tanh(g_a) = 2s-1 is a fused DVE tensor_scalar.
  - c_new = f*c + i*g on DVE; tanh(c_new) per group on ACT; h_new = o*tanh.
"""
import numpy as np
from contextlib import ExitStack

import concourse.bass as bass
import concourse.tile as tile
from concourse import bacc, mybir
from concourse.bass_utils import run_bass_kernel_spmd
from concourse.masks import make_identity

F32 = mybir.dt.float32
F32R = mybir.dt.float32r
AF = mybir.ActivationFunctionType
ALU = mybir.AluOpType

NCORES = 8
BC = 8192            # batch rows per core
GROUP_ROWS = 512     # 4 tiles of 128
NT = 4               # tiles per group
NG = BC // GROUP_ROWS

_CACHE = {}


def _build(has_bias: bool):
    nc = bacc.Bacc("TRN2", target_bir_lowering=False, debug=False)
    x = nc.dram_tensor("x", [BC, 128], F32, kind="ExternalInput").ap()
    h = nc.dram_tensor("h", [BC, 128], F32, kind="ExternalInput").ap()
    c = nc.dram_tensor("c", [BC, 128], F32, kind="ExternalInput").ap()
    wxt = nc.dram_tensor("wxt", [128, 512], F32R, kind="ExternalInput").ap()
    wht = nc.dram_tensor("wht", [128, 512], F32R, kind="ExternalInput").ap()
    if has_bias:
        bias = nc.dram_tensor("bias", [1, 512], F32R, kind="ExternalInput").ap()
    hn = nc.dram_tensor("hn", [BC, 128], F32, kind="ExternalOutput").ap()
    cn = nc.dram_tensor("cn", [BC, 128], F32, kind="ExternalOutput").ap()

    with tile.TileContext(nc) as tc:
        with ExitStack() as ctx:
            const = ctx.enter_context(tc.tile_pool(name="const", bufs=1))
            inp = ctx.enter_context(tc.tile_pool(name="inp", bufs=4))
            xht = ctx.enter_context(tc.tile_pool(name="xht", bufs=8))
            qp = ctx.enter_context(tc.tile_pool(name="qp", bufs=2, space="PSUM"))
            sp = ctx.enter_context(tc.tile_pool(name="sp", bufs=4))
            op = ctx.enter_context(tc.tile_pool(name="op", bufs=6))
            tmp = ctx.enter_context(tc.tile_pool(name="tmp", bufs=6))

            ident = const.tile([128, 128], F32)
            make_identity(nc, ident)
            wx_sb = const.tile([128, 512], F32R)
            nc.sync.dma_start(wx_sb[:], wxt)
            wh_sb = const.tile([128, 512], F32R)
            nc.sync.dma_start(wh_sb[:], wht)
            if has_bias:
                ones = const.tile([1, 128], F32R)
                nc.vector.memset(ones[:], 1.0)
                b_sb = const.tile([1, 512], F32R)
                nc.sync.dma_start(b_sb[:], bias)

            warm = qp.tile([128, 2048], F32, name="warm", tag="quad")
            for _ in range(16):
                nc.tensor.matmul(warm[:, 0:128], ident[:], ident[:],
                                 is_transpose=True, start=True, stop=True)

            xsl = hsl = csl = None
            for g in range(NG):
                r0 = g * GROUP_ROWS
                if g % 2 == 0:
                    # superload: 2 groups (1024 rows, 512KB) per dma_start
                    xsl = inp.tile([128, 2 * GROUP_ROWS], F32,
                                   name=f"xsl{g}", tag="xg")
                    hsl = inp.tile([128, 2 * GROUP_ROWS], F32,
                                   name=f"hsl{g}", tag="hg")
                    csl = inp.tile([128, 2 * GROUP_ROWS], F32,
                                   name=f"csl{g}", tag="cg")
                    for sb_t, dram in ((xsl, x), (hsl, h), (csl, c)):
                        nc.sync.dma_start(
                            sb_t[:].rearrange("p (t f) -> p t f", t=2 * NT),
                            dram[r0:r0 + 2 * GROUP_ROWS, :].rearrange(
                                "(t p) f -> p t f", p=128))
                off = (g % 2) * GROUP_ROWS
                xg = xsl[:, off:off + GROUP_ROWS]
                hg = hsl[:, off:off + GROUP_ROWS]
                cg = csl[:, off:off + GROUP_ROWS]

                quad = qp.tile([128, 2048], F32, name=f"quad{g}", tag="quad")
                # pass A: all transposes back-to-back on PE, then ONE wide
                # rounding cast over all 4 banks (strided 3D AP)
                for t in range(NT):
                    col = t * 512
                    fs = t * 128
                    nc.tensor.matmul(quad[:, col:col + 128],
                                     xg[:, fs:fs + 128], ident[:],
                                     is_transpose=True, start=True, stop=False)
                    nc.tensor.matmul(quad[:, col + 128:col + 256],
                                     hg[:, fs:fs + 128], ident[:],
                                     is_transpose=True, start=False, stop=True)
                xh_w = xht.tile([128, 1024], F32R, name=f"xh{g}", tag="xh")
                nc.vector.tensor_copy(
                    xh_w[:].rearrange("p (t x) -> p t x", t=NT),
                    quad[:].rearrange("p (t x) -> p t x", t=NT)[:, :, 0:256])
                # pass B: all gates matmuls = [x h] @ [WxT; WhT] (+ bias)
                for t in range(NT):
                    col = t * 512
                    xh = xh_w[:, t * 256:(t + 1) * 256]
                    first = True
                    if has_bias:
                        nc.tensor.matmul(quad[:, col:col + 512], ones[:],
                                         b_sb[:], start=True, stop=False)
                        first = False
                    nc.tensor.matmul(quad[:, col:col + 512], xh[:, 0:128],
                                     wx_sb[:], start=first, stop=False)
                    nc.tensor.matmul(quad[:, col:col + 512], xh[:, 128:256],
                                     wh_sb[:], start=False, stop=True)

                sig = sp.tile([128, 2048], F32, name=f"sig{g}", tag="sig")
                cn_g = op.tile([128, GROUP_ROWS], F32, name=f"cn{g}", tag="cn")
                tc_g = op.tile([128, GROUP_ROWS], F32, name=f"tc{g}", tag="tcg")
                hn_g = op.tile([128, GROUP_ROWS], F32, name=f"hn{g}", tag="hn")
                # one sigmoid over the whole quad (4 banks), then wide DVE
                # ops via 3D (p, t, x) access patterns over all 4 tiles.
                nc.scalar.activation(sig[:], quad[:], AF.Sigmoid)
                sig3 = sig[:].rearrange("p (t x) -> p t x", t=NT)
                i_ap = sig3[:, :, 0:128]
                f_ap = sig3[:, :, 128:256]
                o_ap = sig3[:, :, 256:384]
                s_ap = sig3[:, :, 384:512]
                c3 = cg[:].rearrange("p (t x) -> p t x", t=NT)
                gt = tmp.tile([128, 512], F32, name=f"gt{g}", tag="gt")
                gt3 = gt[:].rearrange("p (t x) -> p t x", t=NT)
                nc.vector.tensor_scalar(gt3, s_ap, 2.0, 1.0,
                                        ALU.mult, ALU.subtract)
                ig = tmp.tile([128, 512], F32, name=f"ig{g}", tag="ig")
                ig3 = ig[:].rearrange("p (t x) -> p t x", t=NT)
                nc.vector.tensor_mul(ig3, i_ap, gt3)
                fc = tmp.tile([128, 512], F32, name=f"fc{g}", tag="fc")
                fc3 = fc[:].rearrange("p (t x) -> p t x", t=NT)
                nc.vector.tensor_mul(fc3, f_ap, c3)
                nc.vector.tensor_add(cn_g[:], ig[:], fc[:])
                nc.scalar.activation(tc_g[:], cn_g[:], AF.Tanh)
                tc3 = tc_g[:].rearrange("p (t x) -> p t x", t=NT)
                hn3 = hn_g[:].rearrange("p (t x) -> p t x", t=NT)
                nc.vector.tensor_mul(hn3, o_ap, tc3)
                for sb_t, dram in ((hn_g, hn), (cn_g, cn)):
                    nc.sync.dma_start(
                        dram[r0:r0 + GROUP_ROWS, :].rearrange(
                            "(t p) f -> p t f", p=128),
                        sb_t[:].rearrange("p (t f) -> p t f", t=NT))
    nc.compile()
    return nc


def _run(inputs, trace=False, tmpdir=None):
    x = np.ascontiguousarray(inputs["x"], dtype=np.float32)
    h = np.ascontiguousarray(inputs["h_t"], dtype=np.float32)
    c = np.ascontiguousarray(inputs["c_t"], dtype=np.float32)
    # gate order [i, f, o, g]; W_g/b_g scaled by 2 for the tanh-via-sigmoid
    wx = np.concatenate([inputs["W_ii"], inputs["W_if"], inputs["W_io"],
                         2.0 * np.asarray(inputs["W_ig"])], axis=0)
    wh = np.concatenate([inputs["W_hi"], inputs["W_hf"], inputs["W_ho"],
                         2.0 * np.asarray(inputs["W_hg"])], axis=0)
    b = np.concatenate([inputs["b_i"], inputs["b_f"], inputs["b_o"],
                        2.0 * np.asarray(inputs["b_g"])], axis=0)
    wxt = np.ascontiguousarray(wx.T, dtype=np.float32)
    wht = np.ascontiguousarray(wh.T, dtype=np.float32)
    has_bias = bool(np.any(b))

    key = has_bias
    if key not in _CACHE:
        _CACHE[key] = _build(has_bias)
    nc = _CACHE[key]

    in_maps = []
    for i in range(NCORES):
        m = {
            "x": x[i * BC:(i + 1) * BC],
            "h": h[i * BC:(i + 1) * BC],
            "c": c[i * BC:(i + 1) * BC],
            "wxt": wxt,
            "wht": wht,
        }
        if has_bias:
            m["bias"] = b.reshape(1, 512).astype(np.float32)
        in_maps.append(m)

    res = run_bass_kernel_spmd(nc, in_maps, core_ids=list(range(NCORES)),
                               trace=trace, tmpdir=tmpdir)
    h_new = np.concatenate([r["hn"] for r in res.results], axis=0)
    c_new = np.concatenate([r["cn"] for r in res.results], axis=0)
    return h_new, c_new, res


def kernel(**inputs):
    h_new, c_new, _ = _run(inputs, trace=False)
    return h_new, c_new
